# revision 15
# baseline (speedup 1.0000x reference)
"""Single-head attention (B=8, N=2048, E=1024) on 8 TRN2 NeuronCores.

Sharding: data-parallel over batch — core i computes batch element i fully.

Algebraic reduction (softmax is shift-invariant over the key axis):
  scores = (x Wq^T + bq)(x Wk^T + bk)^T
         ~ x (Wq^T Wk) x^T + 1 (Wk^T bq)^T x^T   (terms constant per query row drop)
so with M = Wq^T Wk and v2 = Wk^T bq precomputed on HOST (f32 BLAS, ~ms):
  g = x M + v2        [N, E]   one projection instead of q AND k
  scores = g x^T * scale
This removes one of the three [N,E]x[E,E] projections: device MACs drop
from 15.0G to 12.9G per core (-14.3%), and the key-side operand of the
scores matmul becomes the raw xT input already resident in SBUF.

Per-core dataflow (all matmul compute in bf16, f32 PSUM accumulation):
  gT[c,n] = M_lhsT.T @ xT_rhs + v2    (projection, c on partitions)
  v[n,e]  = xT_lhsT.T @ WTv_rhs + bv  (natural layout, n on partitions)
  scoresT[j,i] = xT_lhsT.T @ gT_rhs ; expT = exp(scale*scoresT)  (ScalarE)
  denom[i] = ones-matmul over j-partitions of DVE-reduced exp sums
  out[i,e] = (expT_lhsT.T @ v_rhs) * (1/denom)
Softmax skips max-subtraction: scores are ~N(0,1) (max |s| < ~8), exp is
safe in f32 and softmax is shift-invariant.

Startup: the first g-proj chunk runs K-OUTER across 8 simultaneously-open
PSUM banks, so the PE consumes each (w0_k, w1_k, x0_k) slice-trio as the
two HWDGE rings land it (~1.5us/step) instead of idling behind a junk
warmup until the full 2MB arrives. The HAM cold-clock window is spent on
real work.
"""

import numpy as np
import ml_dtypes

P = 128
E = 1024
N = 2048
KO = E // P      # 8 contraction subtiles
NT = N // P      # 16 row tiles
NCH = N // 512   # 4 chunks of 512
SCALE = 0.03125  # 1/sqrt(1024)
WARM = 6         # junk matmuls bridging engine start -> first input tiles

_CACHE = {}


def _build():
    import concourse.bacc as bacc
    import concourse.tile as tile
    import concourse.mybir as mybir

    f32 = mybir.dt.float32
    bf16 = mybir.dt.bfloat16
    AF = mybir.ActivationFunctionType
    Alu = mybir.AluOpType

    nc = bacc.Bacc("TRN2", target_bir_lowering=False, debug=False, num_devices=8)
    xT_d = nc.dram_tensor("xT", [E, N], bf16, kind="ExternalInput")
    WT_d = nc.dram_tensor("WT", [E, 2 * E], bf16, kind="ExternalInput")
    bg_d = nc.dram_tensor("b_g", [P, KO], f32, kind="ExternalInput")
    bv_d = nc.dram_tensor("b_v", [P, E], f32, kind="ExternalInput")
    out_d = nc.dram_tensor("out", [N, E], f32, kind="ExternalOutput")

    xT_r = xT_d.ap().rearrange("(ko p) (c n) -> c p ko n", p=P, n=512)
    WT_r = WT_d.ap().rearrange("(ko p) (s f) -> s p ko f", p=P, f=512)
    out_r = out_d.ap().rearrange("(it p) e -> it p e", p=P)

    with tile.TileContext(nc) as tc:
        with (
            tc.tile_pool(name="const", bufs=1) as const,
            tc.tile_pool(name="qkv", bufs=1) as qkv,
            tc.tile_pool(name="pin", bufs=1) as pin,
        ):
            bg_t = const.tile([P, KO], f32, tag="bg")
            nc.gpsimd.dma_start(bg_t[:], bg_d.ap())
            bv_t = const.tile([P, E], f32, tag="bv")
            nc.gpsimd.dma_start(bv_t[:], bv_d.ap())
            ones_t = const.tile([P, 1], bf16, tag="ones")
            nc.vector.memset(ones_t[:], 1.0)

            # gT split per n-chunk so attention chunk ic only depends on the
            # chunk it reads (finer scheduling deps than one big tile)
            gTc = [
                qkv.tile([P, KO, 512], bf16, tag=f"gT{c}", name=f"gT{c}")
                for c in range(NCH)
            ]
            vt = qkv.tile([P, NT, E], bf16, tag="v")

            # Per-k chunked input tiles. xck[k][c]: x columns c*512..;
            # wck[k][s]: W columns s*512.. (s 0-1: M for the g projection,
            # 2-3: Wv^T). xck stays resident for the whole kernel: it is
            # also the key-side stationary operand of the scores matmuls.
            xck = [[None] * NCH for _ in range(KO)]
            wck = [[None] * 4 for _ in range(KO)]

            def x_tile(c, k):
                t = pin.tile([P, 512], bf16, tag=f"x{k}_{c}", name=f"x{k}_{c}")
                xck[k][c] = t
                return t, xT_r[c][:, k, :]

            def w_tile(s, k):
                t = pin.tile([P, 512], bf16, tag=f"w{k}_{s}", name=f"w{k}_{s}")
                wck[k][s] = t
                return t, WT_r[s][:, k, :]

            # DMA schedule: both HWDGE rings (ACT + SP) carry the k-outer
            # prologue's 3-tile steps (w0_k, w1_k, x0_k = 384KB/step) split
            # 1.5 tiles/ring/step so arrival (~1.5us/step) matches warm PE
            # consumption (1.7us/step). Later tiles queue behind in
            # consumption order. SWDGE (gpsimd) only carries the small bias
            # tensors — bulk loads there are descriptor-gen bound and slow.
            # All tiles in strict consumption order, assigned alternately to
            # the two HWDGE rings (ACT + SP): the packet round-robin then
            # always feeds the front-of-need with both rings' bandwidth.
            # (Dedicating a ring to late tiles starves the other ring's
            # front: that cost v4 a 10.9us x1-wait stall.)
            order = []
            for k in range(KO):            # k-outer prologue streams (w0k, x0k)
                order += [w_tile(0, k), x_tile(0, k)]
            for k in range(KO):
                order.append(w_tile(1, k))
            for c in (1, 2, 3):
                for k in range(KO):
                    order.append(x_tile(c, k))
            for s in (2, 3):
                for k in range(KO):
                    order.append(w_tile(s, k))
            for i, (t, src) in enumerate(order):
                eng = nc.scalar if i % 2 == 0 else nc.sync
                eng.dma_start(t[:], src)

            def x_sl(c, k, fsl):
                return xck[k][c][:, fsl]

            def w_sl(s, k, fsl):
                return wck[k][s][:, fsl]

            with tc.tile_pool(name="pproj", bufs=8, space="PSUM") as pproj:
                # g projection, K-OUTER in HALF-chunks: 4 PSUM banks open per
                # pass, so each k-step consumes one freshly-landed (w_k, x_k)
                # slice pair instead of stalling on a full 1MB chunk, while
                # the 4 ACT evacuations of one half overlap the next half's
                # 6.8us of matmuls (a full-chunk 8-evac burst would bind the
                # next chunk's bank reuse). A few junk matmuls (overwritten
                # by the k=0 start=True pass) bridge engine-start to
                # first-tile-landing and start the HAM clock ramp.
                scratch = pin.tile([P, 512], bf16, tag="warm_in")
                nc.vector.memset(scratch[:], 0.0)
                first = True
                for ch in range(NCH):
                    for hf in range(2):
                        pch = [
                            pproj.tile([P, 512], f32, tag="pp", name=f"pp{ch}_{hf}_{i}")
                            for i in range(4)
                        ]
                        if first:
                            first = False
                            for _ in range(WARM):
                                nc.tensor.matmul(
                                    pch[0][:], lhsT=scratch[:, :P], rhs=scratch[:],
                                    start=True, stop=True,
                                )
                        for k in range(KO):
                            for i in range(4):
                                ft = hf * 4 + i
                                nc.tensor.matmul(
                                    pch[i][:],
                                    lhsT=w_sl(hf, k, slice(i * P, (i + 1) * P)),
                                    rhs=x_sl(ch, k, slice(0, 512)),
                                    start=(k == 0),
                                    stop=(k == KO - 1),
                                )
                        for i in range(4):
                            ft = hf * 4 + i
                            nc.scalar.activation(
                                gTc[ch][:, ft, :], pch[i][:], AF.Identity,
                                bias=bg_t[:, ft : ft + 1], scale=1.0,
                            )

                # v projection -> v [n(part), e]
                for nt in range(NT):
                    for ch2 in range(2):
                        esl = slice(ch2 * 512, (ch2 + 1) * 512)
                        ps = pproj.tile([P, 512], f32, tag="pp")
                        for k in range(KO):
                            nc.tensor.matmul(
                                ps[:],
                                lhsT=x_sl(nt // 4, k, slice((nt % 4) * P, (nt % 4 + 1) * P)),
                                rhs=w_sl(2 + ch2, k, slice(0, 512)),
                                start=(k == 0),
                                stop=(k == KO - 1),
                            )
                        nc.vector.tensor_tensor(
                            out=vt[:, nt, esl],
                            in0=ps[:],
                            in1=bv_t[:, esl],
                            op=Alu.add,
                        )

            with (
                tc.tile_pool(name="attn", bufs=2) as attn,
                tc.tile_pool(name="psc", bufs=2, space="PSUM") as psc,
                tc.tile_pool(name="pnum", bufs=4, space="PSUM") as pnum,
                tc.tile_pool(name="pden", bufs=2, space="PSUM") as pden,
            ):
                # Software pipeline: scores(ic) is emitted before the
                # denominator + numerator of (ic-1), so the DVE exp-sum
                # reduce of chunk ic-1 overlaps with scores matmuls of ic
                # instead of stalling PE.
                def emit_scores(ic):
                    expT = attn.tile([P, NT, 512], bf16, tag="expT", bufs=3)
                    for jt in range(NT):
                        ps = psc.tile([P, 512], f32, tag="ps_s")
                        for k in range(KO):
                            nc.tensor.matmul(
                                ps[:],
                                lhsT=xck[k][jt // 4][:, (jt % 4) * P : (jt % 4 + 1) * P],
                                rhs=gTc[ic][:, k, :],
                                start=(k == 0),
                                stop=(k == KO - 1),
                            )
                        nc.scalar.activation(expT[:, jt, :], ps[:], AF.Exp, scale=SCALE)
                    # softmax denominators, step 1: sum over the 16 j-tiles
                    # (free-dim strided reduce on DVE)
                    sume = attn.tile([P, 512], f32, tag="sume")
                    nc.vector.reduce_sum(
                        sume[:],
                        expT.rearrange("p j i -> p i j"),
                        axis=mybir.AxisListType.X,
                    )
                    # bf16 copy so the cross-partition denominator matmul is a
                    # cheap bf16 op instead of a double-pass fp32 one. On DVE
                    # (not ACT): it waits on the reduce, and ACT's FIFO must
                    # stay clear for the next chunk's EXPs.
                    sume_bf = attn.tile([P, 512], bf16, tag="sume_bf")
                    nc.vector.tensor_copy(sume_bf[:], sume[:])
                    return expT, sume_bf

                def emit_tail(ic, expT, sume):
                    last = ic == NCH - 1
                    for isub in range(4):
                        it = ic * 4 + isub
                        # step 2: sum over the remaining 128 j-partitions
                        psd = pden.tile([P, 1], f32, tag="ps_d")
                        nc.tensor.matmul(
                            psd[:],
                            lhsT=sume[:, isub * P : (isub + 1) * P],
                            rhs=ones_t[:],
                            start=True,
                            stop=True,
                        )
                        rden = attn.tile([P, 1], f32, tag="rden", bufs=4)
                        nc.vector.reciprocal(rden[:], psd[:])
                        osb = attn.tile([P, E], f32, tag="osb", bufs=3)
                        for ch2 in range(2):
                            esl = slice(ch2 * 512, (ch2 + 1) * 512)
                            ps = pnum.tile([P, 512], f32, tag="ps_n")
                            for jt in range(NT):
                                nc.tensor.matmul(
                                    ps[:],
                                    lhsT=expT[:, jt, isub * P : (isub + 1) * P],
                                    rhs=vt[:, jt, esl],
                                    start=(jt == 0),
                                    stop=(jt == NT - 1),
                                )
                            # division on ScalarE (Copy with per-partition
                            # scale) keeps the DVE free so the pden PSUM slot
                            # recycles without stalling the next denom matmul
                            if last and isub == 3:
                                # final tiles: halve the ACT+DMA drain after
                                # the very last matmul
                                for eh in range(2):
                                    hsl = slice(ch2 * 512 + eh * 256,
                                                ch2 * 512 + (eh + 1) * 256)
                                    psl = slice(eh * 256, (eh + 1) * 256)
                                    nc.scalar.activation(
                                        osb[:, hsl], ps[:, psl], AF.Copy,
                                        scale=rden[:],
                                    )
                                    nc.sync.dma_start(out_r[it][:, hsl], osb[:, hsl])
                            else:
                                nc.scalar.activation(
                                    osb[:, esl], ps[:], AF.Copy, scale=rden[:]
                                )
                                nc.sync.dma_start(out_r[it][:, esl], osb[:, esl])

                prev = None
                for ic in range(NCH):
                    cur = emit_scores(ic)
                    if prev is not None:
                        emit_tail(ic - 1, *prev)
                    prev = cur
                emit_tail(NCH - 1, *prev)
    nc.compile()
    return nc


def get_nc():
    if "nc" not in _CACHE:
        _CACHE["nc"] = _build()
    return _CACHE["nc"]


def prepare_in_maps(x, W_qkv, b_qkv):
    bf = ml_dtypes.bfloat16
    x = np.asarray(x, dtype=np.float32)
    W = np.asarray(W_qkv, dtype=np.float32)
    b = np.asarray(b_qkv, dtype=np.float32)
    assert x.shape == (8, N, E) and W.shape == (3 * E, E) and b.shape == (3 * E,)
    Wq, Wk, Wv = W[:E], W[E : 2 * E], W[2 * E :]
    # scores = x (Wq^T Wk) x^T + 1 (Wk^T bq)^T x^T  (softmax-invariant form)
    M = Wq.T.astype(np.float64) @ Wk.astype(np.float64)  # [E, E]
    v2 = Wk.T @ b[:E]  # [E]
    xT = np.ascontiguousarray(np.transpose(x, (0, 2, 1))).astype(bf)  # [8, E, N]
    WT = np.concatenate([M, Wv.T.astype(np.float64)], axis=1).astype(bf)  # [E, 2E]
    bg = np.ascontiguousarray(v2.reshape(KO, P).T)  # [P, KO]
    bv = np.ascontiguousarray(np.broadcast_to(b[2 * E :], (P, E)))  # [P, E]
    return [{"xT": xT[i], "WT": WT, "b_g": bg, "b_v": bv} for i in range(8)]


def kernel(x, W_qkv, b_qkv):
    from concourse.bass_utils import run_bass_kernel_spmd

    nc = get_nc()
    in_maps = prepare_in_maps(x, W_qkv, b_qkv)
    res = run_bass_kernel_spmd(nc, in_maps, core_ids=list(range(8)))
    return np.stack([res.results[i]["out"] for i in range(8)], axis=0)
